# revision 3
# baseline (speedup 1.0000x reference)
"""Causal attention (LN -> QKV -> 16-head causal attn -> out-proj) on 8 TRN2 cores.

Sharding: core c = (batch b=c//4, head-group g=c%4). Each core runs its batch's
LayerNorm + a 4-head slice of QKV / attention / out-projection. The out-proj
partials (column-split over the inner dim) are summed on the host per batch.

v2 schedule (vs baseline):
  - x loads on the sync DMA queue, weight stages on the scalar queue (parallel),
    f32->bf16 weight casts on GpSimd.
  - xn transpose via the DMA XBAR (dma_start(transpose=True)) straight into the
    [dim-part, kb, seq] layout - no PE identity matmuls, no PSUM evacuations.
  - attention runs in q-quarters of 512 with the score PSUM double-buffered so
    S^T(kb+1) overlaps exp(kb) overlaps PV(kb-1); exp is ONE ScalarE call per
    (quarter, pair, kb) covering both heads (FD<=1024 from PSUM).
  - causal diag masks on GpSimd; softmax denominators (the 65th V column trick)
    broadcast via gpsimd.partition_broadcast + one DVE reciprocal - no DRAM hops.
  - the out-projection for quarter qq-1 is emitted inside quarter qq so its
    matmuls/evacuations/DMA hide under the exp-bound attention phase.
"""

import numpy as np

import concourse.bass as bass
import concourse.mybir as mybir
import concourse.tile as tile
from concourse import bacc
from concourse.bass_utils import run_bass_kernel_spmd

B, N, DIM, HEADS, DIM_HEAD = 2, 2048, 1024, 16, 64
INNER = HEADS * DIM_HEAD
H_LOC = 4                      # heads per core
N_CORES = 8
P = 128
NB = N // P                    # 16 seq blocks
KB = DIM // P                  # 8 dim blocks
QQ = 512                       # q span per attention quarter
SCALE = DIM_HEAD ** -0.5
LN_EPS = 1e-5

F32 = mybir.dt.float32
BF16 = mybir.dt.bfloat16
AF = mybir.ActivationFunctionType
ALU = mybir.AluOpType


def build_nc():
    from contextlib import ExitStack

    nc = bacc.Bacc(None, target_bir_lowering=False, debug=False)

    x_d = nc.dram_tensor("x", [N, DIM], F32, kind="ExternalInput")
    wq_d = nc.dram_tensor("wq", [DIM, H_LOC * DIM_HEAD], F32, kind="ExternalInput")
    wk_d = nc.dram_tensor("wk", [DIM, H_LOC * DIM_HEAD], F32, kind="ExternalInput")
    wv_d = nc.dram_tensor("wv", [DIM, H_LOC * DIM_HEAD], F32, kind="ExternalInput")
    wo_d = nc.dram_tensor("wo", [H_LOC * DIM_HEAD, DIM], F32, kind="ExternalInput")
    bq_d = nc.dram_tensor("bq", [P, 2], F32, kind="ExternalInput")
    bk_d = nc.dram_tensor("bk", [P, 2], F32, kind="ExternalInput")
    bv_d = nc.dram_tensor("bv", [1, H_LOC * DIM_HEAD], F32, kind="ExternalInput")
    out_d = nc.dram_tensor("out", [N, DIM], F32, kind="ExternalOutput")

    with tile.TileContext(nc) as tc:
        ctx = ExitStack()
        with ctx:
            const = ctx.enter_context(tc.tile_pool(name="const", bufs=1))
            persist = ctx.enter_context(tc.tile_pool(name="persist", bufs=1))
            wstage = ctx.enter_context(tc.tile_pool(name="wstage", bufs=2))
            xpool = ctx.enter_context(tc.tile_pool(name="xpool", bufs=4))
            xnpool = ctx.enter_context(tc.tile_pool(name="xnpool", bufs=3))
            stat = ctx.enter_context(tc.tile_pool(name="stat", bufs=8))
            expp = ctx.enter_context(tc.tile_pool(name="expp", bufs=3))
            dnp = ctx.enter_context(tc.tile_pool(name="dnp", bufs=2))
            rbcp = ctx.enter_context(tc.tile_pool(name="rbcp", bufs=2))
            dramp = ctx.enter_context(tc.tile_pool(name="dramp", bufs=2, space="DRAM"))
            stage = ctx.enter_context(tc.tile_pool(name="stage", bufs=3))

            # ---- constants ----
            eps_t = const.tile([P, 1], F32, tag="eps")
            nc.vector.memset(eps_t, LN_EPS)
            # keep-mask for the causal diagonal block, both heads stacked in
            # the free dim: tri_b[k, hh, q] = (k <= q)
            tri = const.tile([P, P], BF16, tag="tri")
            nc.gpsimd.memset(tri[:], 0.0)
            nc.gpsimd.affine_select(
                out=tri[:], in_=tri[:], compare_op=ALU.is_gt, fill=1.0,
                base=0, channel_multiplier=1, pattern=[[-1, P]],
            )
            tri_b = const.tile([P, 2, P], BF16, tag="trib")
            nc.gpsimd.tensor_copy(tri_b[:, 0, :], tri[:])
            nc.gpsimd.tensor_copy(tri_b[:, 1, :], tri[:])

            bq_sb = const.tile([P, 2], F32, tag="bq")
            nc.scalar.dma_start(bq_sb[:], bq_d[:])
            bk_sb = const.tile([P, 2], F32, tag="bk")
            nc.scalar.dma_start(bk_sb[:], bk_d[:])
            bv_sb = const.tile([P, H_LOC, DIM_HEAD], F32, tag="bv")
            nc.scalar.dma_start(
                bv_sb[:],
                bv_d[:].rearrange("o (h d) -> o h d", h=H_LOC)
                .to_broadcast((P, H_LOC, DIM_HEAD)),
            )

            # ---- persistent tensors ----
            xnT = [persist.tile([P, KB, 4 * P], BF16, tag=f"xnT{q}", name=f"xnT{q}")
                   for q in range(4)]
            QTt = [persist.tile([P, N], BF16, tag=f"qt{p_}", name=f"qt{p_}")
                   for p_ in range(2)]
            KTt = [persist.tile([P, N], BF16, tag=f"kt{p_}", name=f"kt{p_}")
                   for p_ in range(2)]
            Vt = persist.tile([P, NB, H_LOC, DIM_HEAD + 1], BF16, tag="v")
            nc.gpsimd.memset(Vt[:, :, :, DIM_HEAD:], 1.0)  # denominator column
            outT = [[persist.tile([P, QQ], BF16, tag=f"outT{p_}_{q_}",
                                  name=f"outT{p_}_{q_}") for q_ in range(4)]
                    for p_ in range(2)]

            # ---- weights: stage on the scalar DMA queue, cast on GpSimd ----
            def load_w_bf(dram, shape3, tag, eng):
                st = wstage.tile(shape3, F32, tag="wst", name=f"wst_{tag}")
                nc.scalar.dma_start(
                    st[:], dram[:].rearrange("(kb p) m -> p kb m", p=P)
                )
                bf = persist.tile(shape3, BF16, tag=tag, name=f"bf_{tag}")
                if eng == "dve":
                    nc.vector.tensor_copy(bf[:], st[:])
                else:
                    nc.gpsimd.tensor_copy(bf[:], st[:])
                return bf

            wv_bf = load_w_bf(wv_d, [P, KB, H_LOC * DIM_HEAD], "wv", "dve")
            wq_bf = load_w_bf(wq_d, [P, KB, H_LOC * DIM_HEAD], "wq", "dve")
            wk_bf = load_w_bf(wk_d, [P, KB, H_LOC * DIM_HEAD], "wk", "dve")
            wo_bf = load_w_bf(wo_d, [P, 2, DIM], "wo", "gps")

            # ---- phase A: LN -> DMA-transpose -> QKV/V ----
            ctxA = ExitStack()
            with ctxA:
                psQK = ctxA.enter_context(
                    tc.tile_pool(name="psQK", bufs=4, space="PSUM"))
                psV = ctxA.enter_context(
                    tc.tile_pool(name="psV", bufs=2, space="PSUM"))

                def emit_qkv_st(st):
                    for (wt, bias_sb, dstt) in ((wq_bf, bq_sb, QTt),
                                                (wk_bf, bk_sb, KTt)):
                        for pr in range(2):
                            ps = psQK.tile([P, 512], F32, tag="psqk")
                            for kb in range(KB):
                                nc.tensor.matmul(
                                    ps[:],
                                    wt[:, kb, pr * P:(pr + 1) * P],
                                    xnT[st][:, kb, :],
                                    start=(kb == 0), stop=(kb == KB - 1),
                                )
                            nc.vector.tensor_scalar_add(
                                dstt[pr][:, st * 512:(st + 1) * 512], ps[:],
                                bias_sb[:, pr:pr + 1],
                            )

                for sb in range(NB):
                    st, so = sb // 4, (sb % 4) * P
                    x_t = xpool.tile([P, DIM], F32, tag="x")
                    nc.sync.dma_start(x_t[:], x_d[sb * P:(sb + 1) * P, :])

                    stats = stat.tile([P, 2, 6], F32, tag="bnst")
                    x3 = x_t[:].rearrange("p (a f) -> p a f", a=2)
                    for a in range(2):
                        nc.vector.bn_stats(stats[:, a, :], x3[:, a, :])
                    mv = stat.tile([P, 2], F32, tag="mv")
                    nc.vector.bn_aggr(mv[:], stats[:])
                    rstd = stat.tile([P, 1], F32, tag="rstd")
                    nc.scalar.activation(rstd[:], mv[:, 1:2], AF.Sqrt, bias=eps_t[:])
                    nc.vector.reciprocal(rstd[:], rstd[:])
                    nmrs = stat.tile([P, 1], F32, tag="nmrs")
                    nc.vector.tensor_scalar(
                        nmrs[:], mv[:, 0:1], rstd[:], -1.0, ALU.mult, ALU.mult
                    )
                    xn_bf = xnpool.tile([P, DIM], BF16, tag="xn")
                    nc.scalar.activation(
                        xn_bf[:], x_t[:], AF.Identity, bias=nmrs[:], scale=rstd[:]
                    )

                    # transpose into [dim-part, kb, seq] via the DMA XBAR
                    nc.sync.dma_start(
                        xnT[st][:, :, so:so + P], xn_bf[:], transpose=True
                    )

                    # V for this seq block (natural keys-on-partitions layout)
                    psv = psV.tile([P, 512], F32, tag="psv")
                    for kb in range(KB):
                        nc.tensor.matmul(
                            psv[:, :H_LOC * DIM_HEAD],
                            xnT[st][:, kb, so:so + P],
                            wv_bf[:, kb, :],
                            start=(kb == 0), stop=(kb == KB - 1),
                        )
                    nc.vector.tensor_tensor(
                        Vt[:, sb, :, :DIM_HEAD],
                        psv[:, :H_LOC * DIM_HEAD]
                        .rearrange("p (h d) -> p h d", h=H_LOC),
                        bv_sb[:],
                        ALU.add,
                    )

                    if sb % 4 == 3:
                        emit_qkv_st(st)

            # ---- phase B: attention in q-quarters + interleaved out-proj ----
            ctxB = ExitStack()
            with ctxB:
                psS = ctxB.enter_context(
                    tc.tile_pool(name="psS", bufs=2, space="PSUM"))
                psO = ctxB.enter_context(
                    tc.tile_pool(name="psO", bufs=1, space="PSUM"))
                psP = ctxB.enter_context(
                    tc.tile_pool(name="psP", bufs=2, space="PSUM"))

                def emit_attn(qq, pr):
                    qs = qq * QQ
                    last_kb = 4 * qq + 3
                    ps_o = psO.tile([DIM_HEAD + 1, 2, QQ], F32, tag="pso",
                                    name=f"pso_{qq}_{pr}")
                    for kb in range(last_kb + 1):
                        off = max(0, kb * P - qs)
                        s_ps = psS.tile([P, 2, QQ], F32, tag="sps",
                                        name=f"sps_{qq}_{pr}_{kb}")
                        for hh in range(2):
                            po = hh * DIM_HEAD
                            nc.tensor.matmul(
                                s_ps[:, hh, off:],
                                KTt[pr][po:po + DIM_HEAD, kb * P:(kb + 1) * P],
                                QTt[pr][po:po + DIM_HEAD, qs + off:qs + QQ],
                                start=True, stop=True,
                                tile_position=(po, 0),
                            )
                        ex = expp.tile([P, 2, QQ], BF16, tag="ex",
                                       name=f"ex_{qq}_{pr}_{kb}")
                        nc.scalar.activation(
                            ex[:, :, off:], s_ps[:, :, off:], AF.Exp
                        )
                        if kb * P >= qs:
                            nc.gpsimd.tensor_tensor(
                                ex[:, :, off:off + P],
                                ex[:, :, off:off + P],
                                tri_b[:],
                                ALU.mult,
                            )
                        for hh in range(2):
                            nc.tensor.matmul(
                                ps_o[:, hh, off:],
                                Vt[:, kb, 2 * pr + hh, :],
                                ex[:, hh, off:],
                                start=(kb == 0), stop=(kb == last_kb),
                            )
                    # evacuate + normalize: denominators live in row 64
                    ot = outT[pr][qq]
                    for hh in range(2):
                        nc.vector.tensor_copy(
                            ot[hh * DIM_HEAD:(hh + 1) * DIM_HEAD, :],
                            ps_o[:DIM_HEAD, hh, :],
                        )
                    # denominator reciprocal broadcast via DRAM shuffle:
                    # [1, 1024] row -> [128, 8] lanes -> recip -> DRAM ->
                    # stride-0 broadcast DMA into [128, 512]
                    den = dnp.tile([1, 2, QQ], F32, tag="den",
                                   name=f"den_{qq}_{pr}")
                    nc.vector.tensor_copy(den[:], ps_o[DIM_HEAD:DIM_HEAD + 1, :, :])
                    da = dramp.tile([1, 2 * QQ], F32, tag="da",
                                    name=f"da_{qq}_{pr}")
                    nc.sync.dma_start(da[:], den[:].rearrange("o a q -> o (a q)"))
                    denc = stat.tile([P, 2 * QQ // P], F32, tag="denc",
                                     name=f"denc_{qq}_{pr}")
                    nc.sync.dma_start(
                        denc[:],
                        da[0, :].rearrange("(p o) -> p o", o=2 * QQ // P),
                    )
                    nc.vector.reciprocal(denc[:], denc[:])
                    db = dramp.tile([1, 2 * QQ], F32, tag="db",
                                    name=f"db_{qq}_{pr}")
                    nc.sync.dma_start(
                        db[0, :].rearrange("(p o) -> p o", o=2 * QQ // P),
                        denc[:],
                    )
                    dbc = rbcp.tile([P, QQ], F32, tag="dbc",
                                    name=f"dbc_{qq}_{pr}")
                    for hh in range(2):
                        nc.sync.dma_start(
                            dbc[hh * DIM_HEAD:(hh + 1) * DIM_HEAD, :],
                            db[:, hh * QQ:(hh + 1) * QQ]
                            .to_broadcast((DIM_HEAD, QQ)),
                        )
                    nc.vector.tensor_tensor(ot[:], ot[:], dbc[:], ALU.mult)

                def emit_outproj(qq):
                    for qb in range(4):
                        qrow = qq * QQ + qb * P
                        so_t = stage.tile([P, DIM], F32, tag="so")
                        for nt in range(2):
                            pp = psP.tile([P, 512], F32, tag="pp")
                            for pb in range(2):
                                nc.tensor.matmul(
                                    pp[:],
                                    outT[pb][qq][:, qb * P:(qb + 1) * P],
                                    wo_bf[:, pb, nt * 512:(nt + 1) * 512],
                                    start=(pb == 0), stop=(pb == 1),
                                )
                            nc.vector.tensor_copy(
                                so_t[:, nt * 512:(nt + 1) * 512], pp[:])
                        nc.sync.dma_start(
                            out_d[qrow:qrow + P, :], so_t[:])

                for qq in range(4):
                    emit_attn(qq, 0)
                    if qq > 0:
                        emit_outproj(qq - 1)
                    emit_attn(qq, 1)
                emit_outproj(3)

    nc.compile()
    return nc


def make_in_maps(x, ln_w, ln_b, w_qkv, w_out):
    x = np.asarray(x, np.float32)
    ln_w = np.asarray(ln_w, np.float32)
    ln_b = np.asarray(ln_b, np.float32)
    w_qkv = np.asarray(w_qkv, np.float32)
    w_out = np.asarray(w_out, np.float32)

    in_maps = []
    for c in range(N_CORES):
        b, g = c // 4, c % 4
        cols = np.arange(4 * g * DIM_HEAD, (4 * g + H_LOC) * DIM_HEAD)
        wq_s = w_qkv[:, cols]
        wk_s = w_qkv[:, INNER + cols]
        wv_s = w_qkv[:, 2 * INNER + cols]
        wq = np.ascontiguousarray(ln_w[:, None] * wq_s * SCALE)
        wk = np.ascontiguousarray(ln_w[:, None] * wk_s)
        wv = np.ascontiguousarray(ln_w[:, None] * wv_s)
        bq = (ln_b @ wq_s) * SCALE
        bk = ln_b @ wk_s
        bv = ln_b @ wv_s
        in_maps.append({
            "x": np.ascontiguousarray(x[b]),
            "wq": wq, "wk": wk, "wv": wv,
            "wo": np.ascontiguousarray(w_out[cols, :]),
            "bq": np.ascontiguousarray(bq.reshape(2, P).T),
            "bk": np.ascontiguousarray(bk.reshape(2, P).T),
            "bv": bv.reshape(1, H_LOC * DIM_HEAD),
        })
    return in_maps


_NC_CACHE = []


def kernel(x, ln_w, ln_b, w_qkv, w_out):
    in_maps = make_in_maps(x, ln_w, ln_b, w_qkv, w_out)
    if not _NC_CACHE:
        _NC_CACHE.append(build_nc())
    nc = _NC_CACHE[0]
    res = run_bass_kernel_spmd(nc, in_maps, list(range(N_CORES))).results
    out = np.zeros((B, N, DIM), np.float32)
    for c in range(N_CORES):
        out[c // 4] += res[c]["out"]
    return out


# revision 4
# speedup vs baseline: 1.0762x; 1.0762x over previous
"""Causal attention (LN -> QKV -> 16-head causal attn -> out-proj) on 8 TRN2 cores.

Sharding: core c = (batch b=c//4, head-group g=c%4). Each core runs its batch's
LayerNorm + a 4-head slice of QKV / attention / out-projection. The out-proj
partials (column-split over the inner dim) are summed on the host per batch.

v2 schedule (vs baseline):
  - x loads on the sync DMA queue, weight stages on the scalar queue (parallel),
    f32->bf16 weight casts on GpSimd.
  - xn transpose via the DMA XBAR (dma_start(transpose=True)) straight into the
    [dim-part, kb, seq] layout - no PE identity matmuls, no PSUM evacuations.
  - attention runs in q-quarters of 512 with the score PSUM double-buffered so
    S^T(kb+1) overlaps exp(kb) overlaps PV(kb-1); exp is ONE ScalarE call per
    (quarter, pair, kb) covering both heads (FD<=1024 from PSUM).
  - causal diag masks on GpSimd; softmax denominators (the 65th V column trick)
    broadcast via gpsimd.partition_broadcast + one DVE reciprocal - no DRAM hops.
  - the out-projection for quarter qq-1 is emitted inside quarter qq so its
    matmuls/evacuations/DMA hide under the exp-bound attention phase.
"""

import numpy as np

import concourse.bass as bass
import concourse.mybir as mybir
import concourse.tile as tile
from concourse import bacc
from concourse.bass_utils import run_bass_kernel_spmd

B, N, DIM, HEADS, DIM_HEAD = 2, 2048, 1024, 16, 64
INNER = HEADS * DIM_HEAD
H_LOC = 4                      # heads per core
N_CORES = 8
P = 128
NB = N // P                    # 16 seq blocks
KB = DIM // P                  # 8 dim blocks
QQ = 512                       # q span per attention quarter
SCALE = DIM_HEAD ** -0.5
LN_EPS = 1e-5

F32 = mybir.dt.float32
BF16 = mybir.dt.bfloat16
AF = mybir.ActivationFunctionType
ALU = mybir.AluOpType


def build_nc():
    from contextlib import ExitStack

    nc = bacc.Bacc(None, target_bir_lowering=False, debug=False)

    x_d = nc.dram_tensor("x", [N, DIM], F32, kind="ExternalInput")
    wq_d = nc.dram_tensor("wq", [DIM, H_LOC * DIM_HEAD], F32, kind="ExternalInput")
    wk_d = nc.dram_tensor("wk", [DIM, H_LOC * DIM_HEAD], F32, kind="ExternalInput")
    wv_d = nc.dram_tensor("wv", [DIM, H_LOC * DIM_HEAD], F32, kind="ExternalInput")
    wo_d = nc.dram_tensor("wo", [H_LOC * DIM_HEAD, DIM], F32, kind="ExternalInput")
    bq_d = nc.dram_tensor("bq", [P, 2], F32, kind="ExternalInput")
    bk_d = nc.dram_tensor("bk", [P, 2], F32, kind="ExternalInput")
    bv_d = nc.dram_tensor("bv", [1, H_LOC * DIM_HEAD], F32, kind="ExternalInput")
    out_d = nc.dram_tensor("out", [N, DIM], F32, kind="ExternalOutput")

    with tile.TileContext(nc) as tc:
        ctx = ExitStack()
        with ctx:
            const = ctx.enter_context(tc.tile_pool(name="const", bufs=1))
            persist = ctx.enter_context(tc.tile_pool(name="persist", bufs=1))
            wstage = ctx.enter_context(tc.tile_pool(name="wstage", bufs=2))
            xpool = ctx.enter_context(tc.tile_pool(name="xpool", bufs=4))
            xnpool = ctx.enter_context(tc.tile_pool(name="xnpool", bufs=3))
            stat = ctx.enter_context(tc.tile_pool(name="stat", bufs=8))
            expp = ctx.enter_context(tc.tile_pool(name="expp", bufs=3))
            dnp = ctx.enter_context(tc.tile_pool(name="dnp", bufs=2))
            rbcp = ctx.enter_context(tc.tile_pool(name="rbcp", bufs=2))
            dramp = ctx.enter_context(tc.tile_pool(name="dramp", bufs=2, space="DRAM"))
            stage = ctx.enter_context(tc.tile_pool(name="stage", bufs=3))

            # ---- constants ----
            eps_t = const.tile([P, 1], F32, tag="eps")
            nc.vector.memset(eps_t, LN_EPS)
            # keep-mask for the causal diagonal block, both heads stacked in
            # the free dim: tri_b[k, hh, q] = (k <= q)
            tri = const.tile([P, P], BF16, tag="tri")
            nc.gpsimd.memset(tri[:], 0.0)
            nc.gpsimd.affine_select(
                out=tri[:], in_=tri[:], compare_op=ALU.is_gt, fill=1.0,
                base=0, channel_multiplier=1, pattern=[[-1, P]],
            )
            tri_b = const.tile([P, 2, P], BF16, tag="trib")
            nc.gpsimd.tensor_copy(tri_b[:, 0, :], tri[:])
            nc.gpsimd.tensor_copy(tri_b[:, 1, :], tri[:])

            bq_sb = const.tile([P, 2], F32, tag="bq")
            nc.scalar.dma_start(bq_sb[:], bq_d[:])
            bk_sb = const.tile([P, 2], F32, tag="bk")
            nc.scalar.dma_start(bk_sb[:], bk_d[:])
            bv_sb = const.tile([P, H_LOC, DIM_HEAD], F32, tag="bv")
            nc.scalar.dma_start(
                bv_sb[:],
                bv_d[:].rearrange("o (h d) -> o h d", h=H_LOC)
                .to_broadcast((P, H_LOC, DIM_HEAD)),
            )

            # ---- persistent tensors ----
            xnT = [persist.tile([P, KB, 4 * P], BF16, tag=f"xnT{q}", name=f"xnT{q}")
                   for q in range(4)]
            QTt = [persist.tile([P, N], BF16, tag=f"qt{p_}", name=f"qt{p_}")
                   for p_ in range(2)]
            KTt = [persist.tile([P, N], BF16, tag=f"kt{p_}", name=f"kt{p_}")
                   for p_ in range(2)]
            Vt = persist.tile([P, NB, H_LOC, DIM_HEAD + 1], BF16, tag="v")
            nc.gpsimd.memset(Vt[:, :, :, DIM_HEAD:], 1.0)  # denominator column
            outT = [[persist.tile([P, QQ], BF16, tag=f"outT{p_}_{q_}",
                                  name=f"outT{p_}_{q_}") for q_ in range(4)]
                    for p_ in range(2)]

            # ---- weights: stage on the scalar DMA queue, cast on GpSimd ----
            def load_w_bf(dram, shape3, tag, eng):
                st = wstage.tile(shape3, F32, tag="wst", name=f"wst_{tag}")
                # wo is not needed until the first out-projection: stage it
                # via the gpsimd SWDGE queue so the scalar queue stays free
                # for the xn transposes.
                dma_eng = nc.gpsimd if eng == "gps" else nc.scalar
                dma_eng.dma_start(
                    st[:], dram[:].rearrange("(kb p) m -> p kb m", p=P)
                )
                bf = persist.tile(shape3, BF16, tag=tag, name=f"bf_{tag}")
                if eng == "dve":
                    nc.vector.tensor_copy(bf[:], st[:])
                else:
                    nc.gpsimd.tensor_copy(bf[:], st[:])
                return bf

            wv_bf = load_w_bf(wv_d, [P, KB, H_LOC * DIM_HEAD], "wv", "dve")
            wq_bf = load_w_bf(wq_d, [P, KB, H_LOC * DIM_HEAD], "wq", "dve")
            wk_bf = load_w_bf(wk_d, [P, KB, H_LOC * DIM_HEAD], "wk", "dve")
            wo_bf = load_w_bf(wo_d, [P, 2, DIM], "wo", "gps")

            # ---- phase A: LN -> DMA-transpose -> QKV/V ----
            ctxA = ExitStack()
            with ctxA:
                psQK = ctxA.enter_context(
                    tc.tile_pool(name="psQK", bufs=4, space="PSUM"))
                psV = ctxA.enter_context(
                    tc.tile_pool(name="psV", bufs=2, space="PSUM"))

                def emit_qkv_st(st):
                    for (wt, bias_sb, dstt) in ((wq_bf, bq_sb, QTt),
                                                (wk_bf, bk_sb, KTt)):
                        for pr in range(2):
                            ps = psQK.tile([P, 512], F32, tag="psqk")
                            for kb in range(KB):
                                nc.tensor.matmul(
                                    ps[:],
                                    wt[:, kb, pr * P:(pr + 1) * P],
                                    xnT[st][:, kb, :],
                                    start=(kb == 0), stop=(kb == KB - 1),
                                )
                            nc.vector.tensor_scalar_add(
                                dstt[pr][:, st * 512:(st + 1) * 512], ps[:],
                                bias_sb[:, pr:pr + 1],
                            )

                for sb in range(NB):
                    st, so = sb // 4, (sb % 4) * P
                    x_t = xpool.tile([P, DIM], F32, tag="x")
                    nc.sync.dma_start(x_t[:], x_d[sb * P:(sb + 1) * P, :])

                    stats = stat.tile([P, 2, 6], F32, tag="bnst")
                    x3 = x_t[:].rearrange("p (a f) -> p a f", a=2)
                    for a in range(2):
                        nc.vector.bn_stats(stats[:, a, :], x3[:, a, :])
                    mv = stat.tile([P, 2], F32, tag="mv")
                    nc.vector.bn_aggr(mv[:], stats[:])
                    rstd = stat.tile([P, 1], F32, tag="rstd")
                    nc.scalar.activation(rstd[:], mv[:, 1:2], AF.Sqrt, bias=eps_t[:])
                    nc.vector.reciprocal(rstd[:], rstd[:])
                    nmrs = stat.tile([P, 1], F32, tag="nmrs")
                    nc.vector.tensor_scalar(
                        nmrs[:], mv[:, 0:1], rstd[:], -1.0, ALU.mult, ALU.mult
                    )
                    xn_bf = xnpool.tile([P, DIM], BF16, tag="xn")
                    nc.scalar.activation(
                        xn_bf[:], x_t[:], AF.Identity, bias=nmrs[:], scale=rstd[:]
                    )

                    # transpose into [dim-part, kb, seq] via the DMA XBAR.
                    # Triggered from the scalar queue: the trigger directly
                    # follows this block's xn in ScalarE order, keeping the
                    # sync queue free for the x-load pipeline.
                    nc.scalar.dma_start(
                        xnT[st][:, :, so:so + P], xn_bf[:], transpose=True
                    )

                    # V for this seq block (natural keys-on-partitions layout)
                    psv = psV.tile([P, 512], F32, tag="psv")
                    for kb in range(KB):
                        nc.tensor.matmul(
                            psv[:, :H_LOC * DIM_HEAD],
                            xnT[st][:, kb, so:so + P],
                            wv_bf[:, kb, :],
                            start=(kb == 0), stop=(kb == KB - 1),
                        )
                    nc.vector.tensor_tensor(
                        Vt[:, sb, :, :DIM_HEAD],
                        psv[:, :H_LOC * DIM_HEAD]
                        .rearrange("p (h d) -> p h d", h=H_LOC),
                        bv_sb[:],
                        ALU.add,
                    )

                    if sb % 4 == 3:
                        emit_qkv_st(st)

            # ---- phase B: attention in q-quarters + interleaved out-proj ----
            ctxB = ExitStack()
            with ctxB:
                psS = ctxB.enter_context(
                    tc.tile_pool(name="psS", bufs=2, space="PSUM"))
                psO = ctxB.enter_context(
                    tc.tile_pool(name="psO", bufs=1, space="PSUM"))
                psP = ctxB.enter_context(
                    tc.tile_pool(name="psP", bufs=2, space="PSUM"))

                def emit_attn(qq, pr):
                    qs = qq * QQ
                    last_kb = 4 * qq + 3

                    def emit_sT(kb):
                        off = max(0, kb * P - qs)
                        s_ps = psS.tile([P, 2, QQ], F32, tag="sps",
                                        name=f"sps_{qq}_{pr}_{kb}")
                        for hh in range(2):
                            po = hh * DIM_HEAD
                            nc.tensor.matmul(
                                s_ps[:, hh, off:],
                                KTt[pr][po:po + DIM_HEAD, kb * P:(kb + 1) * P],
                                QTt[pr][po:po + DIM_HEAD, qs + off:qs + QQ],
                                start=True, stop=True,
                                tile_position=(po, 0),
                            )
                        ex = expp.tile([P, 2, QQ], BF16, tag="ex",
                                       name=f"ex_{qq}_{pr}_{kb}")
                        nc.scalar.activation(
                            ex[:, :, off:], s_ps[:, :, off:], AF.Exp
                        )
                        if kb * P >= qs:
                            nc.gpsimd.tensor_tensor(
                                ex[:, :, off:off + P],
                                ex[:, :, off:off + P],
                                tri_b[:],
                                ALU.mult,
                            )
                        return ex, off

                    def emit_pv(kb, ex, off):
                        for hh in range(2):
                            nc.tensor.matmul(
                                ps_o[:, hh, off:],
                                Vt[:, kb, 2 * pr + hh, :],
                                ex[:, hh, off:],
                                start=(kb == 0), stop=(kb == last_kb),
                            )

                    ps_o = psO.tile([DIM_HEAD + 1, 2, QQ], F32, tag="pso",
                                    name=f"pso_{qq}_{pr}")
                    # software-pipelined: S^T runs one kb ahead of PV so the
                    # PE queue never head-of-line blocks on exp
                    pend = None
                    for kb in range(last_kb + 1):
                        cur = (kb, *emit_sT(kb))
                        if pend is not None:
                            emit_pv(*pend)
                        pend = cur
                    emit_pv(*pend)
                    # evacuate + normalize: denominators live in row 64
                    ot = outT[pr][qq]
                    for hh in range(2):
                        nc.vector.tensor_copy(
                            ot[hh * DIM_HEAD:(hh + 1) * DIM_HEAD, :],
                            ps_o[:DIM_HEAD, hh, :],
                        )
                    # denominator reciprocal broadcast via DRAM shuffle:
                    # [1, 1024] row -> [128, 8] lanes -> recip -> DRAM ->
                    # stride-0 broadcast DMA into [128, 512]
                    den = dnp.tile([1, 2, QQ], F32, tag="den",
                                   name=f"den_{qq}_{pr}")
                    nc.vector.tensor_copy(den[:], ps_o[DIM_HEAD:DIM_HEAD + 1, :, :])
                    da = dramp.tile([1, 2 * QQ], F32, tag="da",
                                    name=f"da_{qq}_{pr}")
                    nc.sync.dma_start(da[:], den[:].rearrange("o a q -> o (a q)"))
                    denc = stat.tile([P, 2 * QQ // P], F32, tag="denc",
                                     name=f"denc_{qq}_{pr}")
                    nc.sync.dma_start(
                        denc[:],
                        da[0, :].rearrange("(p o) -> p o", o=2 * QQ // P),
                    )
                    nc.vector.reciprocal(denc[:], denc[:])
                    db = dramp.tile([1, 2 * QQ], F32, tag="db",
                                    name=f"db_{qq}_{pr}")
                    nc.sync.dma_start(
                        db[0, :].rearrange("(p o) -> p o", o=2 * QQ // P),
                        denc[:],
                    )
                    dbc = rbcp.tile([P, QQ], F32, tag="dbc",
                                    name=f"dbc_{qq}_{pr}")
                    for hh in range(2):
                        nc.sync.dma_start(
                            dbc[hh * DIM_HEAD:(hh + 1) * DIM_HEAD, :],
                            db[:, hh * QQ:(hh + 1) * QQ]
                            .to_broadcast((DIM_HEAD, QQ)),
                        )
                    nc.vector.tensor_tensor(ot[:], ot[:], dbc[:], ALU.mult)

                def emit_outproj(qq):
                    for qb in range(4):
                        qrow = qq * QQ + qb * P
                        so_t = stage.tile([P, DIM], F32, tag="so")
                        for nt in range(2):
                            pp = psP.tile([P, 512], F32, tag="pp")
                            for pb in range(2):
                                nc.tensor.matmul(
                                    pp[:],
                                    outT[pb][qq][:, qb * P:(qb + 1) * P],
                                    wo_bf[:, pb, nt * 512:(nt + 1) * 512],
                                    start=(pb == 0), stop=(pb == 1),
                                )
                            nc.vector.tensor_copy(
                                so_t[:, nt * 512:(nt + 1) * 512], pp[:])
                        nc.sync.dma_start(
                            out_d[qrow:qrow + P, :], so_t[:])

                for qq in range(4):
                    emit_attn(qq, 0)
                    if qq > 0:
                        emit_outproj(qq - 1)
                    emit_attn(qq, 1)
                emit_outproj(3)

    nc.compile()
    return nc


def make_in_maps(x, ln_w, ln_b, w_qkv, w_out):
    x = np.asarray(x, np.float32)
    ln_w = np.asarray(ln_w, np.float32)
    ln_b = np.asarray(ln_b, np.float32)
    w_qkv = np.asarray(w_qkv, np.float32)
    w_out = np.asarray(w_out, np.float32)

    in_maps = []
    for c in range(N_CORES):
        b, g = c // 4, c % 4
        cols = np.arange(4 * g * DIM_HEAD, (4 * g + H_LOC) * DIM_HEAD)
        wq_s = w_qkv[:, cols]
        wk_s = w_qkv[:, INNER + cols]
        wv_s = w_qkv[:, 2 * INNER + cols]
        wq = np.ascontiguousarray(ln_w[:, None] * wq_s * SCALE)
        wk = np.ascontiguousarray(ln_w[:, None] * wk_s)
        wv = np.ascontiguousarray(ln_w[:, None] * wv_s)
        bq = (ln_b @ wq_s) * SCALE
        bk = ln_b @ wk_s
        bv = ln_b @ wv_s
        in_maps.append({
            "x": np.ascontiguousarray(x[b]),
            "wq": wq, "wk": wk, "wv": wv,
            "wo": np.ascontiguousarray(w_out[cols, :]),
            "bq": np.ascontiguousarray(bq.reshape(2, P).T),
            "bk": np.ascontiguousarray(bk.reshape(2, P).T),
            "bv": bv.reshape(1, H_LOC * DIM_HEAD),
        })
    return in_maps


_NC_CACHE = []


def kernel(x, ln_w, ln_b, w_qkv, w_out):
    in_maps = make_in_maps(x, ln_w, ln_b, w_qkv, w_out)
    if not _NC_CACHE:
        _NC_CACHE.append(build_nc())
    nc = _NC_CACHE[0]
    res = run_bass_kernel_spmd(nc, in_maps, list(range(N_CORES))).results
    out = np.zeros((B, N, DIM), np.float32)
    for c in range(N_CORES):
        out[c // 4] += res[c]["out"]
    return out


# revision 6
# speedup vs baseline: 1.1828x; 1.0990x over previous
"""Causal attention (LN -> QKV -> 16-head causal attn -> out-proj) on 8 TRN2 cores.

Sharding: core c = (batch b=c//4, head-group g=c%4). Each core runs its batch's
LayerNorm + a 4-head slice of QKV / attention / out-projection. The out-proj
partials (column-split over the inner dim) are summed on the host per batch.

v2 schedule (vs baseline):
  - x loads on the sync DMA queue, weight stages on the scalar queue (parallel),
    f32->bf16 weight casts on GpSimd.
  - xn transpose via the DMA XBAR (dma_start(transpose=True)) straight into the
    [dim-part, kb, seq] layout - no PE identity matmuls, no PSUM evacuations.
  - attention runs in q-quarters of 512 with the score PSUM double-buffered so
    S^T(kb+1) overlaps exp(kb) overlaps PV(kb-1); exp is ONE ScalarE call per
    (quarter, pair, kb) covering both heads (FD<=1024 from PSUM).
  - causal diag masks on GpSimd; softmax denominators (the 65th V column trick)
    broadcast via gpsimd.partition_broadcast + one DVE reciprocal - no DRAM hops.
  - the out-projection for quarter qq-1 is emitted inside quarter qq so its
    matmuls/evacuations/DMA hide under the exp-bound attention phase.
"""

import ml_dtypes
import numpy as np


import concourse.bass as bass
import concourse.mybir as mybir
import concourse.tile as tile
from concourse import bacc
from concourse.bass_utils import run_bass_kernel_spmd

B, N, DIM, HEADS, DIM_HEAD = 2, 2048, 1024, 16, 64
INNER = HEADS * DIM_HEAD
H_LOC = 4                      # heads per core
N_CORES = 8
P = 128
NB = N // P                    # 16 seq blocks
KB = DIM // P                  # 8 dim blocks
QQ = 512                       # q span per attention quarter
SCALE = DIM_HEAD ** -0.5
LN_EPS = 1e-5

F32 = mybir.dt.float32
BF16 = mybir.dt.bfloat16
AF = mybir.ActivationFunctionType
ALU = mybir.AluOpType


def build_nc():
    from contextlib import ExitStack

    nc = bacc.Bacc(None, target_bir_lowering=False, debug=False)

    x_d = nc.dram_tensor("x", [N, DIM], BF16, kind="ExternalInput")
    wq_d = nc.dram_tensor("wq", [DIM, H_LOC * DIM_HEAD], BF16, kind="ExternalInput")
    wk_d = nc.dram_tensor("wk", [DIM, H_LOC * DIM_HEAD], BF16, kind="ExternalInput")
    wv_d = nc.dram_tensor("wv", [DIM, H_LOC * DIM_HEAD], BF16, kind="ExternalInput")
    wo_d = nc.dram_tensor("wo", [H_LOC * DIM_HEAD, DIM], BF16, kind="ExternalInput")
    bq_d = nc.dram_tensor("bq", [P, 2], F32, kind="ExternalInput")
    bk_d = nc.dram_tensor("bk", [P, 2], F32, kind="ExternalInput")
    bv_d = nc.dram_tensor("bv", [1, H_LOC * DIM_HEAD], F32, kind="ExternalInput")
    out_d = nc.dram_tensor("out", [N, DIM], BF16, kind="ExternalOutput")

    with tile.TileContext(nc) as tc:
        ctx = ExitStack()
        with ctx:
            const = ctx.enter_context(tc.tile_pool(name="const", bufs=1))
            persist = ctx.enter_context(tc.tile_pool(name="persist", bufs=1))
            xpool = ctx.enter_context(tc.tile_pool(name="xpool", bufs=6))
            xnpool = ctx.enter_context(tc.tile_pool(name="xnpool", bufs=4))
            stat = ctx.enter_context(tc.tile_pool(name="stat", bufs=8))
            expp = ctx.enter_context(tc.tile_pool(name="expp", bufs=3))
            dnp = ctx.enter_context(tc.tile_pool(name="dnp", bufs=2))
            rbcp = ctx.enter_context(tc.tile_pool(name="rbcp", bufs=2))
            dramp = ctx.enter_context(tc.tile_pool(name="dramp", bufs=2, space="DRAM"))
            stage = ctx.enter_context(tc.tile_pool(name="stage", bufs=3))

            # ---- constants ----
            eps_t = const.tile([P, 1], F32, tag="eps")
            nc.vector.memset(eps_t, LN_EPS)
            # keep-mask for the causal diagonal block, both heads stacked in
            # the free dim: tri_b[k, hh, q] = (k <= q)
            tri = const.tile([P, P], BF16, tag="tri")
            nc.gpsimd.memset(tri[:], 0.0)
            nc.gpsimd.affine_select(
                out=tri[:], in_=tri[:], compare_op=ALU.is_gt, fill=1.0,
                base=0, channel_multiplier=1, pattern=[[-1, P]],
            )
            tri_b = const.tile([P, 2, P], BF16, tag="trib")
            nc.gpsimd.tensor_copy(tri_b[:, 0, :], tri[:])
            nc.gpsimd.tensor_copy(tri_b[:, 1, :], tri[:])

            bq_sb = const.tile([P, 2], F32, tag="bq")
            nc.scalar.dma_start(bq_sb[:], bq_d[:])
            bk_sb = const.tile([P, 2], F32, tag="bk")
            nc.scalar.dma_start(bk_sb[:], bk_d[:])
            bv_sb = const.tile([P, H_LOC, DIM_HEAD], F32, tag="bv")
            nc.scalar.dma_start(
                bv_sb[:],
                bv_d[:].rearrange("o (h d) -> o h d", h=H_LOC)
                .to_broadcast((P, H_LOC, DIM_HEAD)),
            )

            # ---- persistent tensors ----
            xnT = [persist.tile([P, KB, 4 * P], BF16, tag=f"xnT{q}", name=f"xnT{q}")
                   for q in range(4)]
            QTt = [persist.tile([P, N], BF16, tag=f"qt{p_}", name=f"qt{p_}")
                   for p_ in range(2)]
            KTt = [persist.tile([P, N], BF16, tag=f"kt{p_}", name=f"kt{p_}")
                   for p_ in range(2)]
            Vt = persist.tile([P, NB, H_LOC, DIM_HEAD + 1], BF16, tag="v")
            nc.gpsimd.memset(Vt[:, :, :, DIM_HEAD:], 1.0)  # denominator column
            outT = [[persist.tile([P, QQ], BF16, tag=f"outT{p_}_{q_}",
                                  name=f"outT{p_}_{q_}") for q_ in range(4)]
                    for p_ in range(2)]

            # ---- weights: stage on the scalar DMA queue, cast on GpSimd ----
            # weights arrive pre-cast to bf16 from the host: DMA them
            # straight into their SBUF layout. wo rides the gpsimd SWDGE
            # ring (not needed until the first out-projection); the rest go
            # on the scalar ring ahead of the xn transposes.
            def load_w_bf(dram, shape3, tag, eng):
                bf = persist.tile(shape3, BF16, tag=tag, name=f"bf_{tag}")
                dma_eng = nc.gpsimd if eng == "gps" else nc.scalar
                dma_eng.dma_start(
                    bf[:], dram[:].rearrange("(kb p) m -> p kb m", p=P)
                )
                return bf

            wv_bf = load_w_bf(wv_d, [P, KB, H_LOC * DIM_HEAD], "wv", "act")
            wq_bf = load_w_bf(wq_d, [P, KB, H_LOC * DIM_HEAD], "wq", "act")
            wk_bf = load_w_bf(wk_d, [P, KB, H_LOC * DIM_HEAD], "wk", "act")
            wo_bf = load_w_bf(wo_d, [P, 2, DIM], "wo", "gps")

            # ---- phase A: LN -> DMA-transpose -> QKV/V ----
            ctxA = ExitStack()
            with ctxA:
                psQK = ctxA.enter_context(
                    tc.tile_pool(name="psQK", bufs=4, space="PSUM"))
                psV = ctxA.enter_context(
                    tc.tile_pool(name="psV", bufs=2, space="PSUM"))

                def emit_qkv_st(st):
                    for (wt, bias_sb, dstt) in ((wq_bf, bq_sb, QTt),
                                                (wk_bf, bk_sb, KTt)):
                        for pr in range(2):
                            ps = psQK.tile([P, 512], F32, tag="psqk")
                            for kb in range(KB):
                                nc.tensor.matmul(
                                    ps[:],
                                    wt[:, kb, pr * P:(pr + 1) * P],
                                    xnT[st][:, kb, :],
                                    start=(kb == 0), stop=(kb == KB - 1),
                                )
                            nc.vector.tensor_scalar_add(
                                dstt[pr][:, st * 512:(st + 1) * 512], ps[:],
                                bias_sb[:, pr:pr + 1],
                            )

                for sb in range(NB):
                    st, so = sb // 4, (sb % 4) * P
                    x_t = xpool.tile([P, DIM], BF16, tag="x")
                    nc.sync.dma_start(x_t[:], x_d[sb * P:(sb + 1) * P, :])

                    stats = stat.tile([P, 2, 6], F32, tag="bnst")
                    x3 = x_t[:].rearrange("p (a f) -> p a f", a=2)
                    for a in range(2):
                        nc.vector.bn_stats(stats[:, a, :], x3[:, a, :])
                    mv = stat.tile([P, 2], F32, tag="mv")
                    nc.vector.bn_aggr(mv[:], stats[:])
                    rstd = stat.tile([P, 1], F32, tag="rstd")
                    nc.scalar.activation(rstd[:], mv[:, 1:2], AF.Sqrt, bias=eps_t[:])
                    nc.vector.reciprocal(rstd[:], rstd[:])
                    nmrs = stat.tile([P, 1], F32, tag="nmrs")
                    nc.vector.tensor_scalar(
                        nmrs[:], mv[:, 0:1], rstd[:], -1.0, ALU.mult, ALU.mult
                    )
                    xn_bf = xnpool.tile([P, DIM], BF16, tag="xn")
                    nc.scalar.activation(
                        xn_bf[:], x_t[:], AF.Identity, bias=nmrs[:], scale=rstd[:]
                    )

                    # transpose into [dim-part, kb, seq] via the DMA XBAR.
                    # Triggered from the scalar queue: the trigger directly
                    # follows this block's xn in ScalarE order, keeping the
                    # sync queue free for the x-load pipeline.
                    nc.scalar.dma_start(
                        xnT[st][:, :, so:so + P], xn_bf[:], transpose=True
                    )

                    # V for this seq block (natural keys-on-partitions layout)
                    psv = psV.tile([P, 512], F32, tag="psv")
                    for kb in range(KB):
                        nc.tensor.matmul(
                            psv[:, :H_LOC * DIM_HEAD],
                            xnT[st][:, kb, so:so + P],
                            wv_bf[:, kb, :],
                            start=(kb == 0), stop=(kb == KB - 1),
                        )
                    nc.vector.tensor_tensor(
                        Vt[:, sb, :, :DIM_HEAD],
                        psv[:, :H_LOC * DIM_HEAD]
                        .rearrange("p (h d) -> p h d", h=H_LOC),
                        bv_sb[:],
                        ALU.add,
                    )

                    # QKV lagged one quarter: the PE queue never
                    # head-of-line blocks on this quarter's last transpose
                    if sb % 4 == 3 and st > 0:
                        emit_qkv_st(st - 1)
                if True:
                    emit_qkv_st(3)

            # ---- phase B: attention in q-quarters + interleaved out-proj ----
            ctxB = ExitStack()
            with ctxB:
                psS = ctxB.enter_context(
                    tc.tile_pool(name="psS", bufs=2, space="PSUM"))
                psO = ctxB.enter_context(
                    tc.tile_pool(name="psO", bufs=1, space="PSUM"))
                psP = ctxB.enter_context(
                    tc.tile_pool(name="psP", bufs=2, space="PSUM"))

                def emit_outproj_qb(qq, qb):
                    qrow = qq * QQ + qb * P
                    so_t = stage.tile([P, DIM], BF16, tag="so")
                    for nt in range(2):
                        pp = psP.tile([P, 512], F32, tag="pp")
                        for pb in range(2):
                            nc.tensor.matmul(
                                pp[:],
                                outT[pb][qq][:, qb * P:(qb + 1) * P],
                                wo_bf[:, pb, nt * 512:(nt + 1) * 512],
                                start=(pb == 0), stop=(pb == 1),
                            )
                        nc.vector.tensor_copy(
                            so_t[:, nt * 512:(nt + 1) * 512], pp[:])
                    nc.sync.dma_start(out_d[qrow:qrow + P, :], so_t[:])

                def emit_attn(qq, pr, chunks=()):
                    qs = qq * QQ
                    last_kb = 4 * qq + 3

                    def emit_sT(kb):
                        off = max(0, kb * P - qs)
                        s_ps = psS.tile([P, 2, QQ], F32, tag="sps",
                                        name=f"sps_{qq}_{pr}_{kb}")
                        for hh in range(2):
                            po = hh * DIM_HEAD
                            nc.tensor.matmul(
                                s_ps[:, hh, off:],
                                KTt[pr][po:po + DIM_HEAD, kb * P:(kb + 1) * P],
                                QTt[pr][po:po + DIM_HEAD, qs + off:qs + QQ],
                                start=True, stop=True,
                                tile_position=(po, 0),
                            )
                        ex = expp.tile([P, 2, QQ], BF16, tag="ex",
                                       name=f"ex_{qq}_{pr}_{kb}")
                        nc.scalar.activation(
                            ex[:, :, off:], s_ps[:, :, off:], AF.Exp
                        )
                        if kb * P >= qs:
                            nc.gpsimd.tensor_tensor(
                                ex[:, :, off:off + P],
                                ex[:, :, off:off + P],
                                tri_b[:],
                                ALU.mult,
                            )
                        return ex, off

                    def emit_pv(kb, ex, off):
                        for hh in range(2):
                            nc.tensor.matmul(
                                ps_o[:, hh, off:],
                                Vt[:, kb, 2 * pr + hh, :],
                                ex[:, hh, off:],
                                start=(kb == 0), stop=(kb == last_kb),
                            )

                    ps_o = psO.tile([DIM_HEAD + 1, 2, QQ], F32, tag="pso",
                                    name=f"pso_{qq}_{pr}")
                    # software-pipelined: S^T runs one kb ahead of PV so the
                    # PE queue never head-of-line blocks on exp; out-proj
                    # chunks for the previous quarter are sprinkled through
                    # the loop to keep the PE warm during exp-bound stretches
                    chunks = list(chunks)
                    cpos = {(i + 1) * (last_kb + 1) // (len(chunks) + 1): i
                            for i in range(len(chunks))} if chunks else {}
                    pend = None
                    for kb in range(last_kb + 1):
                        cur = (kb, *emit_sT(kb))
                        if pend is not None:
                            emit_pv(*pend)
                        if kb in cpos:
                            emit_outproj_qb(*chunks[cpos[kb]])
                        pend = cur
                    emit_pv(*pend)
                    # evacuate + normalize: denominators live in row 64
                    # denominator row first: it feeds the longest chain
                    # (the DRAM-shuffle reciprocal broadcast)
                    den = dnp.tile([1, 2, QQ], F32, tag="den",
                                   name=f"den_{qq}_{pr}")
                    nc.vector.tensor_copy(den[:], ps_o[DIM_HEAD:DIM_HEAD + 1, :, :])
                    ot = outT[pr][qq]
                    for hh in range(2):
                        nc.vector.tensor_copy(
                            ot[hh * DIM_HEAD:(hh + 1) * DIM_HEAD, :],
                            ps_o[:DIM_HEAD, hh, :],
                        )
                    da = dramp.tile([1, 2 * QQ], F32, tag="da",
                                    name=f"da_{qq}_{pr}")
                    nc.sync.dma_start(da[:], den[:].rearrange("o a q -> o (a q)"))
                    denc = stat.tile([P, 2 * QQ // P], F32, tag="denc",
                                     name=f"denc_{qq}_{pr}")
                    nc.sync.dma_start(
                        denc[:],
                        da[0, :].rearrange("(p o) -> p o", o=2 * QQ // P),
                    )
                    nc.vector.reciprocal(denc[:], denc[:])
                    db = dramp.tile([1, 2 * QQ], F32, tag="db",
                                    name=f"db_{qq}_{pr}")
                    nc.sync.dma_start(
                        db[0, :].rearrange("(p o) -> p o", o=2 * QQ // P),
                        denc[:],
                    )
                    dbc = rbcp.tile([P, QQ], F32, tag="dbc",
                                    name=f"dbc_{qq}_{pr}")
                    for hh in range(2):
                        nc.sync.dma_start(
                            dbc[hh * DIM_HEAD:(hh + 1) * DIM_HEAD, :],
                            db[:, hh * QQ:(hh + 1) * QQ]
                            .to_broadcast((DIM_HEAD, QQ)),
                        )
                    nc.vector.tensor_tensor(ot[:], ot[:], dbc[:], ALU.mult)

                for qq in range(4):
                    ch0 = [(qq - 1, 0), (qq - 1, 1)] if qq > 0 else []
                    ch1 = [(qq - 1, 2), (qq - 1, 3)] if qq > 0 else []
                    emit_attn(qq, 0, ch0)
                    emit_attn(qq, 1, ch1)
                for qb in range(4):
                    emit_outproj_qb(3, qb)

    nc.compile()
    return nc


def make_in_maps(x, ln_w, ln_b, w_qkv, w_out):
    x = np.asarray(x, np.float32)
    ln_w = np.asarray(ln_w, np.float32)
    ln_b = np.asarray(ln_b, np.float32)
    w_qkv = np.asarray(w_qkv, np.float32)
    w_out = np.asarray(w_out, np.float32)

    in_maps = []
    for c in range(N_CORES):
        b, g = c // 4, c % 4
        cols = np.arange(4 * g * DIM_HEAD, (4 * g + H_LOC) * DIM_HEAD)
        wq_s = w_qkv[:, cols]
        wk_s = w_qkv[:, INNER + cols]
        wv_s = w_qkv[:, 2 * INNER + cols]
        wq = np.ascontiguousarray(ln_w[:, None] * wq_s * SCALE)
        wk = np.ascontiguousarray(ln_w[:, None] * wk_s)
        wv = np.ascontiguousarray(ln_w[:, None] * wv_s)
        bq = (ln_b @ wq_s) * SCALE
        bk = ln_b @ wk_s
        bv = ln_b @ wv_s
        bf = ml_dtypes.bfloat16
        in_maps.append({
            "x": np.ascontiguousarray(x[b].astype(bf)),
            "wq": wq.astype(bf), "wk": wk.astype(bf), "wv": wv.astype(bf),
            "wo": np.ascontiguousarray(w_out[cols, :]).astype(bf),
            "bq": np.ascontiguousarray(bq.reshape(2, P).T),
            "bk": np.ascontiguousarray(bk.reshape(2, P).T),
            "bv": bv.reshape(1, H_LOC * DIM_HEAD),
        })
    return in_maps


_NC_CACHE = []


def kernel(x, ln_w, ln_b, w_qkv, w_out):
    in_maps = make_in_maps(x, ln_w, ln_b, w_qkv, w_out)
    if not _NC_CACHE:
        _NC_CACHE.append(build_nc())
    nc = _NC_CACHE[0]
    res = run_bass_kernel_spmd(nc, in_maps, list(range(N_CORES))).results
    out = np.zeros((B, N, DIM), np.float32)
    for c in range(N_CORES):
        out[c // 4] += np.asarray(res[c]["out"], dtype=np.float32)
    return out


# revision 7
# speedup vs baseline: 1.2420x; 1.0501x over previous
"""Causal attention (LN -> QKV -> 16-head causal attn -> out-proj) on 8 TRN2 cores.

Sharding: core c = (batch b=c//4, head-group g=c%4). Each core runs its batch's
LayerNorm + a 4-head slice of QKV / attention / out-projection. The out-proj
partials (column-split over the inner dim) are summed on the host per batch.

v2 schedule (vs baseline):
  - x loads on the sync DMA queue, weight stages on the scalar queue (parallel),
    f32->bf16 weight casts on GpSimd.
  - xn transpose via the DMA XBAR (dma_start(transpose=True)) straight into the
    [dim-part, kb, seq] layout - no PE identity matmuls, no PSUM evacuations.
  - attention runs in q-quarters of 512 with the score PSUM double-buffered so
    S^T(kb+1) overlaps exp(kb) overlaps PV(kb-1); exp is ONE ScalarE call per
    (quarter, pair, kb) covering both heads (FD<=1024 from PSUM).
  - causal diag masks on GpSimd; softmax denominators (the 65th V column trick)
    broadcast via gpsimd.partition_broadcast + one DVE reciprocal - no DRAM hops.
  - the out-projection for quarter qq-1 is emitted inside quarter qq so its
    matmuls/evacuations/DMA hide under the exp-bound attention phase.
"""

import ml_dtypes
import numpy as np


import concourse.bass as bass
import concourse.mybir as mybir
import concourse.tile as tile
from concourse import bacc
from concourse.bass_utils import run_bass_kernel_spmd

B, N, DIM, HEADS, DIM_HEAD = 2, 2048, 1024, 16, 64
INNER = HEADS * DIM_HEAD
H_LOC = 4                      # heads per core
N_CORES = 8
P = 128
NB = N // P                    # 16 seq blocks
KB = DIM // P                  # 8 dim blocks
QQ = 512                       # q span per attention quarter
SCALE = DIM_HEAD ** -0.5
LN_EPS = 1e-5

F32 = mybir.dt.float32
BF16 = mybir.dt.bfloat16
AF = mybir.ActivationFunctionType
ALU = mybir.AluOpType


def build_nc():
    from contextlib import ExitStack

    nc = bacc.Bacc(None, target_bir_lowering=False, debug=False)

    x_d = nc.dram_tensor("x", [N, DIM], BF16, kind="ExternalInput")
    wq_d = nc.dram_tensor("wq", [DIM, H_LOC * DIM_HEAD], BF16, kind="ExternalInput")
    wk_d = nc.dram_tensor("wk", [DIM, H_LOC * DIM_HEAD], BF16, kind="ExternalInput")
    wv_d = nc.dram_tensor("wv", [DIM, H_LOC * DIM_HEAD], BF16, kind="ExternalInput")
    wo_d = nc.dram_tensor("wo", [H_LOC * DIM_HEAD, DIM], BF16, kind="ExternalInput")
    bq_d = nc.dram_tensor("bq", [P, 2], F32, kind="ExternalInput")
    bk_d = nc.dram_tensor("bk", [P, 2], F32, kind="ExternalInput")
    bv_d = nc.dram_tensor("bv", [1, H_LOC * DIM_HEAD], F32, kind="ExternalInput")
    out_d = nc.dram_tensor("out", [N, DIM], BF16, kind="ExternalOutput")

    with tile.TileContext(nc) as tc:
        ctx = ExitStack()
        with ctx:
            const = ctx.enter_context(tc.tile_pool(name="const", bufs=1))
            persist = ctx.enter_context(tc.tile_pool(name="persist", bufs=1))
            xpool = ctx.enter_context(tc.tile_pool(name="xpool", bufs=6))
            xnpool = ctx.enter_context(tc.tile_pool(name="xnpool", bufs=4))
            stat = ctx.enter_context(tc.tile_pool(name="stat", bufs=8))
            expp = ctx.enter_context(tc.tile_pool(name="expp", bufs=3))
            dnp = ctx.enter_context(tc.tile_pool(name="dnp", bufs=2))
            rbcp = ctx.enter_context(tc.tile_pool(name="rbcp", bufs=2))
            dramp = ctx.enter_context(tc.tile_pool(name="dramp", bufs=2, space="DRAM"))
            stage = ctx.enter_context(tc.tile_pool(name="stage", bufs=3))

            # ---- constants ----
            eps_t = const.tile([P, 1], F32, tag="eps")
            nc.vector.memset(eps_t, LN_EPS)
            # keep-mask for the causal diagonal block, both heads stacked in
            # the free dim: tri_b[k, hh, q] = (k <= q)
            tri = const.tile([P, P], BF16, tag="tri")
            nc.gpsimd.memset(tri[:], 0.0)
            nc.gpsimd.affine_select(
                out=tri[:], in_=tri[:], compare_op=ALU.is_gt, fill=1.0,
                base=0, channel_multiplier=1, pattern=[[-1, P]],
            )
            tri_b = const.tile([P, 2, P], BF16, tag="trib")
            nc.gpsimd.tensor_copy(tri_b[:, 0, :], tri[:])
            nc.gpsimd.tensor_copy(tri_b[:, 1, :], tri[:])

            bq_sb = const.tile([P, 2], F32, tag="bq")
            nc.scalar.dma_start(bq_sb[:], bq_d[:])
            bk_sb = const.tile([P, 2], F32, tag="bk")
            nc.scalar.dma_start(bk_sb[:], bk_d[:])
            bv_sb = const.tile([P, H_LOC, DIM_HEAD], F32, tag="bv")
            nc.scalar.dma_start(
                bv_sb[:],
                bv_d[:].rearrange("o (h d) -> o h d", h=H_LOC)
                .to_broadcast((P, H_LOC, DIM_HEAD)),
            )

            # ---- persistent tensors ----
            xnT = [persist.tile([P, KB, 4 * P], BF16, tag=f"xnT{q}", name=f"xnT{q}")
                   for q in range(4)]
            QTt = [persist.tile([P, N], BF16, tag=f"qt{p_}", name=f"qt{p_}")
                   for p_ in range(2)]
            KTt = [persist.tile([P, N], BF16, tag=f"kt{p_}", name=f"kt{p_}")
                   for p_ in range(2)]
            Vt = persist.tile([P, NB, H_LOC, DIM_HEAD + 1], BF16, tag="v")
            nc.gpsimd.memset(Vt[:, :, :, DIM_HEAD:], 1.0)  # denominator column
            outT = [[persist.tile([P, QQ], BF16, tag=f"outT{p_}_{q_}",
                                  name=f"outT{p_}_{q_}") for q_ in range(4)]
                    for p_ in range(2)]

            # ---- weights: stage on the scalar DMA queue, cast on GpSimd ----
            # weights arrive pre-cast to bf16 from the host: DMA them
            # straight into their SBUF layout. wo rides the gpsimd SWDGE
            # ring (not needed until the first out-projection); the rest go
            # on the scalar ring ahead of the xn transposes.
            def load_w_bf(dram, shape3, tag, eng):
                bf = persist.tile(shape3, BF16, tag=tag, name=f"bf_{tag}")
                dma_eng = nc.gpsimd if eng == "gps" else nc.scalar
                dma_eng.dma_start(
                    bf[:], dram[:].rearrange("(kb p) m -> p kb m", p=P)
                )
                return bf

            wv_bf = load_w_bf(wv_d, [P, KB, H_LOC * DIM_HEAD], "wv", "act")
            wq_bf = load_w_bf(wq_d, [P, KB, H_LOC * DIM_HEAD], "wq", "act")
            wk_bf = load_w_bf(wk_d, [P, KB, H_LOC * DIM_HEAD], "wk", "act")
            wo_bf = load_w_bf(wo_d, [P, 2, DIM], "wo", "gps")

            # ---- pools: QKV/V/out-proj share one 2-bank psum pool;
            # scores double-buffered (4 banks); PV accumulator 2 banks ----
            psA = ctx.enter_context(tc.tile_pool(name="psA", bufs=2, space="PSUM"))
            psS = ctx.enter_context(tc.tile_pool(name="psS", bufs=2, space="PSUM"))
            psO = ctx.enter_context(tc.tile_pool(name="psO", bufs=1, space="PSUM"))

            def emit_qkv_st(st):
                for (wt, bias_sb, dstt) in ((wq_bf, bq_sb, QTt),
                                            (wk_bf, bk_sb, KTt)):
                    for pr in range(2):
                        ps = psA.tile([P, 512], F32, tag="pa")
                        for kb in range(KB):
                            nc.tensor.matmul(
                                ps[:],
                                wt[:, kb, pr * P:(pr + 1) * P],
                                xnT[st][:, kb, :],
                                start=(kb == 0), stop=(kb == KB - 1),
                            )
                        nc.vector.tensor_scalar_add(
                            dstt[pr][:, st * 512:(st + 1) * 512], ps[:],
                            bias_sb[:, pr:pr + 1],
                        )

            def emit_blocks(st):
                for sb in range(4 * st, 4 * st + 4):
                    so = (sb % 4) * P
                    x_t = xpool.tile([P, DIM], BF16, tag="x")
                    nc.sync.dma_start(x_t[:], x_d[sb * P:(sb + 1) * P, :])

                    stats = stat.tile([P, 2, 6], F32, tag="bnst")
                    x3 = x_t[:].rearrange("p (a f) -> p a f", a=2)
                    for a in range(2):
                        nc.vector.bn_stats(stats[:, a, :], x3[:, a, :])
                    mv = stat.tile([P, 2], F32, tag="mv")
                    nc.vector.bn_aggr(mv[:], stats[:])
                    rstd = stat.tile([P, 1], F32, tag="rstd")
                    nc.scalar.activation(rstd[:], mv[:, 1:2], AF.Sqrt, bias=eps_t[:])
                    nc.vector.reciprocal(rstd[:], rstd[:])
                    nmrs = stat.tile([P, 1], F32, tag="nmrs")
                    nc.vector.tensor_scalar(
                        nmrs[:], mv[:, 0:1], rstd[:], -1.0, ALU.mult, ALU.mult
                    )
                    xn_bf = xnpool.tile([P, DIM], BF16, tag="xn")
                    nc.scalar.activation(
                        xn_bf[:], x_t[:], AF.Identity, bias=nmrs[:], scale=rstd[:]
                    )

                    # transpose into [dim-part, kb, seq] via the DMA XBAR;
                    # triggered from the scalar queue right after this
                    # block's xn so the sync queue stays free for x loads
                    nc.scalar.dma_start(
                        xnT[st][:, :, so:so + P], xn_bf[:], transpose=True
                    )

                    # V for this seq block (natural keys-on-partitions layout)
                    psv = psA.tile([P, 512], F32, tag="pa")
                    for kb in range(KB):
                        nc.tensor.matmul(
                            psv[:, :H_LOC * DIM_HEAD],
                            xnT[st][:, kb, so:so + P],
                            wv_bf[:, kb, :],
                            start=(kb == 0), stop=(kb == KB - 1),
                        )
                    nc.vector.tensor_tensor(
                        Vt[:, sb, :, :DIM_HEAD],
                        psv[:, :H_LOC * DIM_HEAD]
                        .rearrange("p (h d) -> p h d", h=H_LOC),
                        bv_sb[:],
                        ALU.add,
                    )

            def emit_outproj_qb(qq, qb):
                qrow = qq * QQ + qb * P
                so_t = stage.tile([P, DIM], BF16, tag="so")
                for nt in range(2):
                    pp = psA.tile([P, 512], F32, tag="pa")
                    for pb in range(2):
                        nc.tensor.matmul(
                            pp[:],
                            outT[pb][qq][:, qb * P:(qb + 1) * P],
                            wo_bf[:, pb, nt * 512:(nt + 1) * 512],
                            start=(pb == 0), stop=(pb == 1),
                        )
                    nc.vector.tensor_copy(
                        so_t[:, nt * 512:(nt + 1) * 512], pp[:])
                nc.sync.dma_start(out_d[qrow:qrow + P, :], so_t[:])

            def emit_attn(qq, pr, chunks=()):
                qs = qq * QQ
                last_kb = 4 * qq + 3

                def emit_sT(kb):
                    off = max(0, kb * P - qs)
                    s_ps = psS.tile([P, 2, QQ], F32, tag="sps",
                                    name=f"sps_{qq}_{pr}_{kb}")
                    for hh in range(2):
                        po = hh * DIM_HEAD
                        nc.tensor.matmul(
                            s_ps[:, hh, off:],
                            KTt[pr][po:po + DIM_HEAD, kb * P:(kb + 1) * P],
                            QTt[pr][po:po + DIM_HEAD, qs + off:qs + QQ],
                            start=True, stop=True,
                            tile_position=(po, 0),
                        )
                    ex = expp.tile([P, 2, QQ], BF16, tag="ex",
                                   name=f"ex_{qq}_{pr}_{kb}")
                    nc.scalar.activation(
                        ex[:, :, off:], s_ps[:, :, off:], AF.Exp
                    )
                    if kb * P >= qs:
                        nc.gpsimd.tensor_tensor(
                            ex[:, :, off:off + P],
                            ex[:, :, off:off + P],
                            tri_b[:],
                            ALU.mult,
                        )
                    return ex, off

                def emit_pv(kb, ex, off):
                    for hh in range(2):
                        nc.tensor.matmul(
                            ps_o[:, hh, off:],
                            Vt[:, kb, 2 * pr + hh, :],
                            ex[:, hh, off:],
                            start=(kb == 0), stop=(kb == last_kb),
                        )

                ps_o = psO.tile([DIM_HEAD + 1, 2, QQ], F32, tag="pso",
                                name=f"pso_{qq}_{pr}")
                # software-pipelined: S^T runs one kb ahead of PV so the PE
                # queue never head-of-line blocks on exp; out-proj chunks of
                # finished quarters sprinkle through the loop to keep the PE
                # warm during exp-bound stretches
                chunks = list(chunks)
                cpos = {(i + 1) * (last_kb + 1) // (len(chunks) + 1): i
                        for i in range(len(chunks))} if chunks else {}
                pend = None
                for kb in range(last_kb + 1):
                    cur = (kb, *emit_sT(kb))
                    if pend is not None:
                        emit_pv(*pend)
                    if kb in cpos:
                        emit_outproj_qb(*chunks[cpos[kb]])
                    pend = cur
                emit_pv(*pend)

                # denominator row first: it feeds the longest chain
                # (the DRAM-shuffle reciprocal broadcast)
                den = dnp.tile([1, 2, QQ], F32, tag="den",
                               name=f"den_{qq}_{pr}")
                nc.vector.tensor_copy(den[:], ps_o[DIM_HEAD:DIM_HEAD + 1, :, :])
                ot = outT[pr][qq]
                for hh in range(2):
                    nc.vector.tensor_copy(
                        ot[hh * DIM_HEAD:(hh + 1) * DIM_HEAD, :],
                        ps_o[:DIM_HEAD, hh, :],
                    )
                da = dramp.tile([1, 2 * QQ], F32, tag="da",
                                name=f"da_{qq}_{pr}")
                nc.sync.dma_start(da[:], den[:].rearrange("o a q -> o (a q)"))
                denc = stat.tile([P, 2 * QQ // P], F32, tag="denc",
                                 name=f"denc_{qq}_{pr}")
                nc.sync.dma_start(
                    denc[:],
                    da[0, :].rearrange("(p o) -> p o", o=2 * QQ // P),
                )
                nc.vector.reciprocal(denc[:], denc[:])
                db = dramp.tile([1, 2 * QQ], F32, tag="db",
                                name=f"db_{qq}_{pr}")
                nc.sync.dma_start(
                    db[0, :].rearrange("(p o) -> p o", o=2 * QQ // P),
                    denc[:],
                )
                dbc = rbcp.tile([P, QQ], F32, tag="dbc",
                                name=f"dbc_{qq}_{pr}")
                for hh in range(2):
                    nc.sync.dma_start(
                        dbc[hh * DIM_HEAD:(hh + 1) * DIM_HEAD, :],
                        db[:, hh * QQ:(hh + 1) * QQ]
                        .to_broadcast((DIM_HEAD, QQ)),
                    )
                nc.vector.tensor_tensor(ot[:], ot[:], dbc[:], ALU.mult)

            # ---- fully interleaved schedule ----
            # ScalarE order: LN0 LN1 exp(q0) LN2 exp(q1a) LN3 exp(q1b) ...
            # keeps LN ahead of the growing exp batches; attention quarter
            # qq runs as soon as QKV(qq) is out.
            emit_blocks(0)
            emit_blocks(1)
            emit_qkv_st(0)
            emit_attn(0, 0)
            emit_attn(0, 1)
            emit_blocks(2)
            emit_qkv_st(1)
            emit_blocks(3)
            emit_attn(1, 0)
            emit_attn(1, 1)
            emit_qkv_st(2)
            emit_attn(2, 0, [(0, 0), (0, 1)])
            emit_attn(2, 1, [(0, 2), (0, 3)])
            emit_qkv_st(3)
            emit_attn(3, 0, [(1, 0), (1, 1), (1, 2), (1, 3)])
            emit_attn(3, 1, [(2, 0), (2, 1), (2, 2), (2, 3)])
            for qb in range(4):
                emit_outproj_qb(3, qb)

    nc.compile()
    return nc


def make_in_maps(x, ln_w, ln_b, w_qkv, w_out):
    x = np.asarray(x, np.float32)
    ln_w = np.asarray(ln_w, np.float32)
    ln_b = np.asarray(ln_b, np.float32)
    w_qkv = np.asarray(w_qkv, np.float32)
    w_out = np.asarray(w_out, np.float32)

    in_maps = []
    for c in range(N_CORES):
        b, g = c // 4, c % 4
        cols = np.arange(4 * g * DIM_HEAD, (4 * g + H_LOC) * DIM_HEAD)
        wq_s = w_qkv[:, cols]
        wk_s = w_qkv[:, INNER + cols]
        wv_s = w_qkv[:, 2 * INNER + cols]
        wq = np.ascontiguousarray(ln_w[:, None] * wq_s * SCALE)
        wk = np.ascontiguousarray(ln_w[:, None] * wk_s)
        wv = np.ascontiguousarray(ln_w[:, None] * wv_s)
        bq = (ln_b @ wq_s) * SCALE
        bk = ln_b @ wk_s
        bv = ln_b @ wv_s
        bf = ml_dtypes.bfloat16
        in_maps.append({
            "x": np.ascontiguousarray(x[b].astype(bf)),
            "wq": wq.astype(bf), "wk": wk.astype(bf), "wv": wv.astype(bf),
            "wo": np.ascontiguousarray(w_out[cols, :]).astype(bf),
            "bq": np.ascontiguousarray(bq.reshape(2, P).T),
            "bk": np.ascontiguousarray(bk.reshape(2, P).T),
            "bv": bv.reshape(1, H_LOC * DIM_HEAD),
        })
    return in_maps


_NC_CACHE = []


def kernel(x, ln_w, ln_b, w_qkv, w_out):
    in_maps = make_in_maps(x, ln_w, ln_b, w_qkv, w_out)
    if not _NC_CACHE:
        _NC_CACHE.append(build_nc())
    nc = _NC_CACHE[0]
    res = run_bass_kernel_spmd(nc, in_maps, list(range(N_CORES))).results
    out = np.zeros((B, N, DIM), np.float32)
    for c in range(N_CORES):
        out[c // 4] += np.asarray(res[c]["out"], dtype=np.float32)
    return out
